# revision 18
# baseline (speedup 1.0000x reference)
"""GQA attention (B=2, T=2048, D=2048, 32 heads / 8 KV groups, head_dim=64,
RoPE, causal) distributed over 8 TRN2 NeuronCores.

Sharding: core i handles batch b = i//4 and KV-group pair (2*(i%4), 2*(i%4)+1),
i.e. 8 query heads + 2 KV heads. QKV is column-sharded, out-proj row-sharded;
each core writes a partial [T, D] output (bf16) and the host sums 4 partials
per batch. No collectives.

v3 design notes (vs the 355us v2):
 - scores are two CONCURRENT row-tiled K=64 matmuls (h0 on PE rows 0-63, h1 on
   rows 64-127, kp/q stored head-stacked) -> half the score cycles, no q padding
 - softmax exp (ACT, ~1us/tile) is hidden by a global work queue: qkv/proj
   matmul quanta are pulled between each score and PV emission, so the PE
   always has ~2k cycles of independent work while ACT catches up
 - all qkv chunks are front-loaded (qkv(s+1) fills attention window s) so the
   last chunk's exp batch starts as early as possible
 - ramp: 8 warmup matmuls on a memset tile un-throttle the HAM clock during
   the initial DMA wait; wq/xt/wp are host-packed to [128, k, n] so they load
   in a handful of large DMAs instead of 205 small ones
 - denominators of both heads evacuate in one DVE copy + one DMA; out tiles
   are [128, 2048] so each token block stores with a single DMA
"""

import sys

sys.path.insert(0, "/opt/trn_rl_repo")

from collections import deque
from contextlib import ExitStack

import numpy as np
import ml_dtypes

from concourse import bacc, mybir, tile
from concourse.bass_utils import run_bass_kernel_spmd

# problem constants (hardcoded per contract)
B, T, D = 2, 2048, 2048
N_HEAD, N_GROUPS, HEAD_DIM = 32, 8, 64
KV_DIM = N_GROUPS * HEAD_DIM  # 512
NCORES = 8
WCOLS = 768  # 512 q + 128 k + 128 v per core

F32 = mybir.dt.float32
BF16 = mybir.dt.bfloat16
TQ = 512  # token chunk
NT = T // TQ  # 4
NCT = D // 128  # 16 contraction tiles for QKV
SCALE = float(HEAD_DIM) ** -0.5


# ---------------------------------------------------------------- host tables


def _host_tables():
    theta = 1.0 / (10000.0 ** (np.arange(0, HEAD_DIM, 2, dtype=np.float64) / HEAD_DIM))
    freqs = np.arange(T, dtype=np.float64)[None, :] * theta[:, None]  # [32, T]
    cos64 = np.repeat(np.cos(freqs), 2, axis=0)  # rows 2i,2i+1 -> cos_i
    sin64 = np.repeat(np.sin(freqs), 2, axis=0)
    sgn = np.where(np.arange(HEAD_DIM) % 2 == 0, -1.0, 1.0)[:, None]
    cos128 = np.concatenate([cos64, cos64], 0)  # [128, T]
    sin128 = np.concatenate([sin64 * sgn, sin64 * sgn], 0)

    swp = np.zeros((128, 128), np.float32)  # swap(q)[d] = q[d^1]
    for d in range(128):
        swp[d ^ 1, d] = 1.0

    kt = np.arange(128)[:, None]
    qt = np.arange(128)[None, :]
    umask = (qt >= kt).astype(np.float32)  # [kt, qt] causal keep-mask
    umask2 = np.stack([umask, umask], axis=1)  # [128, 2, 128] (both heads)

    selb = np.zeros((128, 128), np.float32)  # va_g[kt,d] = v_sb[64g+d, kt]
    for d in range(64):
        selb[d, d] = 1.0  # cols 0-63: group 0
        selb[64 + d, 64 + d] = 1.0  # cols 64-127: group 1

    # sel4[jj]: bcast rows 2jj / 2jj+1 -> psum rows 0-63 / 64-127
    sel4 = np.zeros((4, 128, 128), np.float32)
    for jj in range(4):
        sel4[jj, 2 * jj, :64] = 1.0
        sel4[jj, 2 * jj + 1, 64:] = 1.0
    bf = ml_dtypes.bfloat16
    return (cos128.astype(bf), sin128.astype(bf), swp.astype(bf), umask2.astype(bf),
            selb.astype(bf), sel4.astype(bf))


def _shard_inputs(x, w_qkv, w_proj):
    """Per-core input dicts. Core i: batch i//4, group pair gp = i%4."""
    cos128, sin128, swp, umask2, selb, sel4 = _host_tables()
    bf = ml_dtypes.bfloat16
    # xt packed [128, 16, T]: xtp[p, ci, t] = x[b].T[128*ci+p, t]
    xtp = []
    for b in range(B):
        xt = x[b].T.astype(bf)  # [D, T]
        xtp.append(np.ascontiguousarray(xt.reshape(NCT, 128, T).transpose(1, 0, 2)))
    maps = []
    for i in range(NCORES):
        b, gp = i // 4, i % 4
        heads = [8 * gp + j for j in range(8)]  # global heads of this core
        # q blocks pair local heads (j, j+4) = (group 2gp head j, group 2gp+1 head j)
        qcols = []
        for j in range(4):
            qcols.append(w_qkv[:, 64 * heads[j] : 64 * heads[j] + 64])
            qcols.append(w_qkv[:, 64 * heads[j + 4] : 64 * heads[j + 4] + 64])
        kcol = w_qkv[:, D + 128 * gp : D + 128 * gp + 128]
        vcol = w_qkv[:, D + KV_DIM + 128 * gp : D + KV_DIM + 128 * gp + 128]
        wq = np.concatenate(qcols + [kcol, vcol], axis=1).astype(bf)  # [D, 768]
        wqp = np.ascontiguousarray(wq.reshape(NCT, 128, WCOLS).transpose(1, 0, 2))
        # w_proj rows in ypair order: pair j = [head j ; head j+4]
        wrows = []
        for j in range(4):
            wrows.append(w_proj[64 * heads[j] : 64 * heads[j] + 64, :])
            wrows.append(w_proj[64 * heads[j + 4] : 64 * heads[j + 4] + 64, :])
        wp = np.concatenate(wrows, axis=0).astype(bf)  # [512, D]
        wpp = np.ascontiguousarray(wp.reshape(4, 128, D).transpose(1, 0, 2))
        maps.append(
            {
                "xt": xtp[b],
                "wqkv": wqp,
                "wproj": wpp,
                "costab": cos128,
                "sintab": sin128,
                "swp": swp,
                "umask2": umask2,
                "selb": selb,
                "sel4": sel4,
            }
        )
    return maps


# ------------------------------------------------------------------- builder


def build_nc():
    nc = bacc.Bacc("TRN2", target_bir_lowering=False, debug=False, num_devices=NCORES)
    xt_d = nc.dram_tensor("xt", [128, NCT, T], BF16, kind="ExternalInput").ap()
    wq_d = nc.dram_tensor("wqkv", [128, NCT, WCOLS], BF16, kind="ExternalInput").ap()
    wp_d = nc.dram_tensor("wproj", [128, 4, D], BF16, kind="ExternalInput").ap()
    cos_d = nc.dram_tensor("costab", [128, T], BF16, kind="ExternalInput").ap()
    sin_d = nc.dram_tensor("sintab", [128, T], BF16, kind="ExternalInput").ap()
    swp_d = nc.dram_tensor("swp", [128, 128], BF16, kind="ExternalInput").ap()
    um2_d = nc.dram_tensor("umask2", [128, 2, 128], BF16, kind="ExternalInput").ap()
    slb_d = nc.dram_tensor("selb", [128, 128], BF16, kind="ExternalInput").ap()
    s4_d = nc.dram_tensor("sel4", [4, 128, 128], BF16, kind="ExternalInput").ap()
    out_d = nc.dram_tensor("out", [T, D], BF16, kind="ExternalOutput").ap()

    with (
        nc.allow_low_precision(reason="bf16 matmul operands; fp32 psum accumulation"),
        tile.TileContext(nc) as tc,
        ExitStack() as ctx,
    ):
        const = ctx.enter_context(tc.tile_pool(name="const", bufs=1))
        keep = ctx.enter_context(tc.tile_pool(name="keep", bufs=1))
        p_x = ctx.enter_context(tc.tile_pool(name="p_x", bufs=2))
        p_w = ctx.enter_context(tc.tile_pool(name="p_w", bufs=3))
        p_yh = ctx.enter_context(tc.tile_pool(name="p_yh", bufs=3))
        p_pt = ctx.enter_context(tc.tile_pool(name="p_pt", bufs=5))
        ps_a = ctx.enter_context(tc.tile_pool(name="ps_a", bufs=2, space="PSUM"))
        ps_sc = ctx.enter_context(tc.tile_pool(name="ps_sc", bufs=2, space="PSUM"))
        ps_pv = ctx.enter_context(tc.tile_pool(name="ps_pv", bufs=1, space="PSUM"))

        warm_t = const.tile([128, TQ], BF16)
        cos_t = const.tile([128, T], BF16)
        sin_t = const.tile([128, T], BF16)
        swp_t = const.tile([128, 128], BF16)
        um2_t = const.tile([128, 2, 128], BF16)
        slb_t = const.tile([128, 128], BF16)
        s4_t = const.tile([128, 4, 128], BF16)
        wq_sb = keep.tile([128, NCT, WCOLS], BF16, tag="wq", name="wq_sb")
        wp_sb = keep.tile([128, 4, D], BF16, tag="wp", name="wp_sb")

        # persistent per-chunk activations; q pairs head-stacked [h0; h1]
        qp_c = [
            [keep.tile([128, TQ], BF16, tag=f"qp{c}_{jp}", name=f"qp{c}_{jp}") for jp in range(4)]
            for c in range(NT)
        ]
        kp_c = [keep.tile([128, TQ], BF16, tag=f"kp{c}", name=f"kp{c}") for c in range(NT)]
        v_c = [keep.tile([128, TQ], BF16, tag=f"v{c}", name=f"v{c}") for c in range(NT)]
        va_c = [
            [keep.tile([128, 4, 65], BF16, tag=f"va{c}_{g}", name=f"va{c}_{g}") for g in range(2)]
            for c in range(NT)
        ]
        yp_c = [
            [keep.tile([128, TQ], BF16, tag=f"yp{c}_{jj}", name=f"yp{c}_{jj}") for jj in range(4)]
            for c in range(NT)
        ]
        sump_c = [keep.tile([128, TQ], BF16, tag=f"sump{c}", name=f"sump{c}") for c in range(NT)]

        # sump rows 8-127 are never written but are read by the bcast matmul
        # (times zero stationary cols); they must be finite. va cols 64 are the
        # static ones-columns (denominator trick); warm_t feeds warmup matmuls.
        nc.gpsimd.memset(warm_t[:], 0.0)
        for c in range(NT):
            nc.gpsimd.memset(sump_c[c][:], 1.0)
            for g in range(2):
                nc.gpsimd.memset(va_c[c][g][:, :, 64:65], 1.0)

        xts = [None] * NT  # xt sbuf tiles [128, NCT, TQ] per chunk

        # ---------------- work queue: qkv (hi) / proj (lo) matmul quanta
        q_hi = deque()
        q_lo = deque()

        def pull(n, lo_ok=True):
            for _ in range(n):
                if q_hi:
                    q_hi.popleft()()
                elif lo_ok and q_lo:
                    q_lo.popleft()()
                else:
                    break

        def drain_hi():
            while q_hi:
                q_hi.popleft()()

        # ---------------- DMA helpers (host-packed layouts, few big DMAs)
        def dma_xt(c, eng=None):
            xt_t = p_x.tile([128, NCT, TQ], BF16, tag="xt", name=f"xt{c}")
            (eng or nc.sync).dma_start(xt_t[:], xt_d[:, :, TQ * c : TQ * c + TQ])
            xts[c] = xt_t

        # ---------------- qkv + rope
        def rope_rest(c, raw, dest):
            ts = slice(TQ * c, TQ * c + TQ)
            sw = ps_a.tile([128, TQ], F32, tag="acc", name="sw")
            nc.tensor.matmul(sw[:], swp_t[:], raw[:], start=True, stop=True)
            t1 = p_w.tile([128, TQ], BF16, tag="t1", name="t1")
            t2 = p_w.tile([128, TQ], BF16, tag="t2", name="t2")
            nc.vector.tensor_mul(t1[:], raw[:], cos_t[:, ts])
            nc.vector.tensor_mul(t2[:], sw[:], sin_t[:, ts])
            nc.vector.tensor_add(dest[:], t1[:], t2[:])

        def qkv_mms(c, oc, lo, hi, ps):
            for ci in range(lo, hi):
                nc.tensor.matmul(
                    ps[:],
                    wq_sb[:, ci, 128 * oc : 128 * oc + 128],
                    xts[c][:, ci, :],
                    start=(ci == 0),
                    stop=(ci == NCT - 1),
                )

        def qkv_post(c, oc, hold):
            if oc < 5:
                rope_rest(c, hold["raw"], qp_c[c][oc] if oc < 4 else kp_c[c])
            else:
                for kloc in range(4):
                    vp = ps_a.tile([128, 128], F32, tag="acc", name="vp")
                    nc.tensor.matmul(
                        vp[:],
                        v_c[c][:, 128 * kloc : 128 * kloc + 128],
                        slb_t[:],
                        start=True,
                        stop=True,
                    )
                    nc.vector.tensor_copy(va_c[c][0][:, kloc, 0:64], vp[:, 0:64])
                    nc.vector.tensor_copy(va_c[c][1][:, kloc, 0:64], vp[:, 64:128])

        def enqueue_qkv(c):
            # per oc: 4 quanta of 4 accumulation mms (the last one also starts
            # the PSUM evacuation), then a separate quantum for the rope / v
            # post-processing so its PE ops (swap matmul, vp) land a quantum
            # later than the DVE copy they wait on. The psum tile is created
            # inside the FIRST quantum so ps_a slot-rotation order matches
            # emission order (no WAR deadlocks).
            for oc in [4, 5, 0, 1, 2, 3]:
                hold = {}
                for lo in range(0, NCT, 4):
                    def quant(c=c, oc=oc, lo=lo, hold=hold):
                        if lo == 0:
                            hold["ps"] = ps_a.tile(
                                [128, TQ], F32, tag="acc", name=f"qkv{c}_{oc}"
                            )
                        qkv_mms(c, oc, lo, lo + 4, hold["ps"])
                        if lo + 4 == NCT:
                            if oc == 5:
                                nc.vector.tensor_copy(v_c[c][:], hold["ps"][:])
                            else:
                                raw = p_w.tile([128, TQ], BF16, tag="raw", name="raw")
                                nc.vector.tensor_copy(raw[:], hold["ps"][:])
                                hold["raw"] = raw
                    q_hi.append(quant)
                q_hi.append(lambda c=c, oc=oc, hold=hold: qkv_post(c, oc, hold))

        # ---------------- attention
        def emit_sc(s, jp, kj):
            qcs, kloc = kj // 4, kj % 4
            col0 = max(kj * 128 - s * TQ, 0)
            sc = ps_sc.tile([128, 2, TQ], F32, tag="sc", name="sc")
            for h in range(2):
                nc.tensor.matmul(
                    sc[:, h, col0:TQ],
                    kp_c[qcs][64 * h : 64 * h + 64, 128 * kloc : 128 * kloc + 128],
                    qp_c[s][jp][64 * h : 64 * h + 64, col0:TQ],
                    start=True,
                    stop=True,
                )
            pt = p_pt.tile([128, 2, TQ], BF16, tag="pt", name="pt")
            nc.scalar.activation(
                pt[:, :, col0:TQ],
                sc[:, :, col0:TQ],
                mybir.ActivationFunctionType.Exp,
                scale=SCALE,
            )
            if kj >= 4 * s:  # diagonal tile: triangular keep-mask, both heads
                nc.gpsimd.tensor_mul(
                    pt[:, :, col0 : col0 + 128],
                    pt[:, :, col0 : col0 + 128],
                    um2_t[:],
                )
            return pt, col0

        def emit_pv(s, jp, kj, pv, pt, col0):
            qcs, kloc = kj // 4, kj % 4
            nkj = 4 * s + 4
            for h in range(2):
                nc.tensor.matmul(
                    pv[0:65, h, col0:TQ],
                    va_c[qcs][h][:, kloc, :],
                    pt[:, h, col0:TQ],
                    start=(kj == 0),
                    stop=(kj == nkj - 1),
                )

        def attn_evac(s, jp, pv):
            # y of h0 -> yp rows 0-63 directly; h1 via staging + partition-move
            # DMA; both denominators in one copy + one DMA into sump rows.
            nc.vector.tensor_copy(yp_c[s][jp][0:64, :], pv[0:64, 0, :])
            yh = p_yh.tile([64, TQ], BF16, tag="yh", name="yh")
            nc.vector.tensor_copy(yh[0:64, :], pv[0:64, 1, :])
            nc.sync.dma_start(yp_c[s][jp][64:128, :], yh[0:64, :])
            dn = p_yh.tile([65, 2, TQ], BF16, tag="dn", name="dn")
            nc.vector.tensor_copy(dn[64:65, :, :], pv[64:65, :, :])
            nc.sync.dma_start(sump_c[s][2 * jp : 2 * jp + 2, :], dn[64:65, :, :])

        cr = [0.0]  # fractional filler-pull credit, persists across jp

        def attn(s, jp, pace, lo_ok):
            nkj = 4 * s + 4
            pv = ps_pv.tile([128, 2, TQ], F32, tag="pv", name="pv")

            def paced_pull():
                cr[0] += pace
                n = int(cr[0])
                cr[0] -= n
                pull(n, lo_ok)

            prev = emit_sc(s, jp, 0)
            for kj in range(1, nkj):
                cur = emit_sc(s, jp, kj)
                paced_pull()
                emit_pv(s, jp, kj - 1, pv, *prev)
                prev = cur
            paced_pull()
            emit_pv(s, jp, nkj - 1, pv, *prev)
            attn_evac(s, jp, pv)

        # ---------------- normalization + projection
        def norm_recip(c):
            # 1/x = exp(-ln(x)) on ACT: Ln and Exp share the natural_log_exp
            # table set, and this keeps the 8*512 reciprocal off the DVE
            # (DVE reciprocal is 8 cyc/elem ~ 3.3us; this is 2 ACT passes)
            lg = p_w.tile([8, TQ], F32, tag="lg", name="lg")
            nc.scalar.activation(
                lg[0:8, :], sump_c[c][0:8, :], mybir.ActivationFunctionType.Ln
            )
            nc.scalar.activation(
                sump_c[c][0:8, :], lg[0:8, :],
                mybir.ActivationFunctionType.Exp, scale=-1.0,
            )

        def norm_bcast(c):
            for jj in range(4):
                bc = ps_a.tile([128, TQ], F32, tag="acc", name="bc")
                nc.tensor.matmul(
                    bc[:], s4_t[:, jj, :], sump_c[c][:], start=True, stop=True
                )
                nc.vector.tensor_mul(yp_c[c][jj][:], yp_c[c][jj][:], bc[:])

        def proj_unit(c, tb, oc, hold):
            if oc == 0:
                hold["ot"] = p_w.tile(
                    [128, D], BF16, tag="ot", name=f"ot{c}_{tb}", bufs=3
                )
            ot = hold["ot"]
            pj = ps_a.tile([128, TQ], F32, tag="acc", name="pj")
            for jj in range(4):
                nc.tensor.matmul(
                    pj[:],
                    yp_c[c][jj][:, 128 * tb : 128 * tb + 128],
                    wp_sb[:, jj, TQ * oc : TQ * oc + TQ],
                    start=(jj == 0),
                    stop=(jj == 3),
                )
            tok0 = c * TQ + tb * 128
            if c == NT - 1:
                # epilogue: ACT is idle -> alternate evac engines, store per oc
                # so the out DMAs drain while the remaining proj mms still run
                if oc % 2 == 1:
                    nc.scalar.copy(ot[:, TQ * oc : TQ * oc + TQ], pj[:])
                else:
                    nc.vector.tensor_copy(ot[:, TQ * oc : TQ * oc + TQ], pj[:])
                nc.sync.dma_start(
                    out_d[tok0 : tok0 + 128, TQ * oc : TQ * oc + TQ],
                    ot[:, TQ * oc : TQ * oc + TQ],
                )
            else:
                nc.vector.tensor_copy(ot[:, TQ * oc : TQ * oc + TQ], pj[:])
                if oc == 3:
                    nc.sync.dma_start(out_d[tok0 : tok0 + 128, :], ot[:])

        def enqueue_proj(c):
            for tb in range(4):
                hold = {}
                for oc in range(4):
                    q_lo.append(
                        lambda c=c, tb=tb, oc=oc, hold=hold: proj_unit(c, tb, oc, hold)
                    )

        # ---------------- emission
        # warmup matmuls: un-throttle the HAM clock during the initial DMA wait
        wps = ps_a.tile([128, TQ], F32, tag="acc", name="warm")
        for wi in range(8):
            nc.tensor.matmul(
                wps[:], warm_t[:, 0:128], warm_t[:], start=(wi == 0), stop=(wi == 7)
            )

        # critical DMAs first: wq quarters on the sync queue, xt(0) quarters
        # on the scalar queue (second HWDGE ring) so they stream in parallel
        for g in range(4):
            nc.sync.dma_start(wq_sb[:, 4 * g : 4 * g + 4, :], wq_d[:, 4 * g : 4 * g + 4, :])
            xt_t = (
                p_x.tile([128, NCT, TQ], BF16, tag="xt", name="xt0") if g == 0 else xts[0]
            )
            xts[0] = xt_t
            nc.scalar.dma_start(
                xt_t[:, 4 * g : 4 * g + 4, :], xt_d[:, 4 * g : 4 * g + 4, 0:TQ]
            )
        dma_xt(1, eng=nc.scalar)
        nc.sync.dma_start(cos_t[:], cos_d)
        nc.sync.dma_start(sin_t[:], sin_d)
        nc.sync.dma_start(swp_t[:], swp_d)
        nc.sync.dma_start(um2_t[:], um2_d)
        nc.sync.dma_start(slb_t[:], slb_d)
        for jj in range(4):
            nc.sync.dma_start(s4_t[:, jj, :], s4_d[jj, :, :])
        nc.scalar.dma_start(wp_sb[:], wp_d)

        # qkv(0) emitted densely (DMA-paced anyway)
        enqueue_qkv(0)
        drain_hi()

        for s in range(NT):
            drain_hi()  # qkv(s) leftovers must finish before attn(s)
            if s + 2 < NT:
                dma_xt(s + 2)  # after drain: its p_x slot's old readers are emitted
            if s + 1 < NT:
                enqueue_qkv(s + 1)
            if s >= 1:
                norm_recip(s - 1)
            # filler supply this window: qkv(s+1) quanta (+ proj(s-1) units,
            # except in window 2, where proj is deferred to window 3 to match
            # window 3's large exp batch); spread evenly over attention tiles
            lo_ok = s != 2
            navail = len(q_hi) + len(q_lo)
            if s >= 1 and lo_ok:
                navail += 16  # proj(s-1), enqueued after attn(s, 0)
            if not lo_ok:
                navail = len(q_hi)
            if s == NT - 1:
                navail = max(0, navail - 6)  # reserve PE work for the recip gap
            pace = navail / (4 * (4 * s + 4))
            for jp in range(4):
                attn(s, jp, pace, lo_ok)
                if s >= 1 and jp == 0:
                    norm_bcast(s - 1)
                    enqueue_proj(s - 1)
        norm_recip(NT - 1)
        while q_lo:
            q_lo.popleft()()
        norm_bcast(NT - 1)
        enqueue_proj(NT - 1)
        while q_lo:
            q_lo.popleft()()

    nc.compile()
    return nc


_NC_CACHE = None


def _get_nc():
    global _NC_CACHE
    if _NC_CACHE is None:
        _NC_CACHE = build_nc()
    return _NC_CACHE


def kernel(x, w_qkv, w_proj, _trace=False, _nc=None):
    x = np.asarray(x, np.float32)
    w_qkv = np.asarray(w_qkv, np.float32)
    w_proj = np.asarray(w_proj, np.float32)
    nc = _nc if _nc is not None else _get_nc()
    in_maps = _shard_inputs(x, w_qkv, w_proj)
    res = run_bass_kernel_spmd(nc, in_maps, core_ids=list(range(NCORES)), trace=_trace)
    out = np.zeros((B, T, D), np.float32)
    for i in range(NCORES):
        out[i // 4] += res.results[i]["out"].astype(np.float32)
    if _trace:
        return out, res
    return out


if __name__ == "__main__":
    rng = np.random.default_rng(0)
    x = rng.standard_normal((B, T, D), dtype=np.float32)
    wq = rng.standard_normal((D, D + 2 * KV_DIM), dtype=np.float32) * D**-0.5
    wp = rng.standard_normal((D, D), dtype=np.float32) * D**-0.5
    y = kernel(x, wq, wp)
    print(y.shape, y.dtype)


# revision 20
# speedup vs baseline: 1.0168x; 1.0168x over previous
"""GQA attention (B=2, T=2048, D=2048, 32 heads / 8 KV groups, head_dim=64,
RoPE, causal) distributed over 8 TRN2 NeuronCores.

Sharding: core i handles batch b = i//4 and KV-group pair (2*(i%4), 2*(i%4)+1),
i.e. 8 query heads + 2 KV heads. QKV is column-sharded, out-proj row-sharded;
each core writes a partial [T, D] output (bf16) and the host sums 4 partials
per batch. No collectives.

v3 design notes (vs the 355us v2):
 - scores are two CONCURRENT row-tiled K=64 matmuls (h0 on PE rows 0-63, h1 on
   rows 64-127, kp/q stored head-stacked) -> half the score cycles, no q padding
 - softmax exp (ACT, ~1us/tile) is hidden by a global work queue: qkv/proj
   matmul quanta are pulled between each score and PV emission, so the PE
   always has ~2k cycles of independent work while ACT catches up
 - all qkv chunks are front-loaded (qkv(s+1) fills attention window s) so the
   last chunk's exp batch starts as early as possible
 - ramp: 8 warmup matmuls on a memset tile un-throttle the HAM clock during
   the initial DMA wait; wq/xt/wp are host-packed to [128, k, n] so they load
   in a handful of large DMAs instead of 205 small ones
 - denominators of both heads evacuate in one DVE copy + one DMA; out tiles
   are [128, 2048] so each token block stores with a single DMA
"""

import sys

sys.path.insert(0, "/opt/trn_rl_repo")

from collections import deque
from contextlib import ExitStack

import numpy as np
import ml_dtypes

from concourse import bacc, mybir, tile
from concourse.bass_utils import run_bass_kernel_spmd

# problem constants (hardcoded per contract)
B, T, D = 2, 2048, 2048
N_HEAD, N_GROUPS, HEAD_DIM = 32, 8, 64
KV_DIM = N_GROUPS * HEAD_DIM  # 512
NCORES = 8
WCOLS = 768  # 512 q + 128 k + 128 v per core

F32 = mybir.dt.float32
BF16 = mybir.dt.bfloat16
TQ = 512  # token chunk
NT = T // TQ  # 4
NCT = D // 128  # 16 contraction tiles for QKV
SCALE = float(HEAD_DIM) ** -0.5


# ---------------------------------------------------------------- host tables


def _host_tables():
    theta = 1.0 / (10000.0 ** (np.arange(0, HEAD_DIM, 2, dtype=np.float64) / HEAD_DIM))
    freqs = np.arange(T, dtype=np.float64)[None, :] * theta[:, None]  # [32, T]
    cos64 = np.repeat(np.cos(freqs), 2, axis=0)  # rows 2i,2i+1 -> cos_i
    sin64 = np.repeat(np.sin(freqs), 2, axis=0)
    sgn = np.where(np.arange(HEAD_DIM) % 2 == 0, -1.0, 1.0)[:, None]
    cos128 = np.concatenate([cos64, cos64], 0)  # [128, T]
    sin128 = np.concatenate([sin64 * sgn, sin64 * sgn], 0)

    swp = np.zeros((128, 128), np.float32)  # swap(q)[d] = q[d^1]
    for d in range(128):
        swp[d ^ 1, d] = 1.0

    kt = np.arange(128)[:, None]
    qt = np.arange(128)[None, :]
    umask = (qt >= kt).astype(np.float32)  # [kt, qt] causal keep-mask
    umask2 = np.stack([umask, umask], axis=1)  # [128, 2, 128] (both heads)

    selb = np.zeros((128, 128), np.float32)  # va_g[kt,d] = v_sb[64g+d, kt]
    for d in range(64):
        selb[d, d] = 1.0  # cols 0-63: group 0
        selb[64 + d, 64 + d] = 1.0  # cols 64-127: group 1

    # sel4[jj]: bcast rows 2jj / 2jj+1 -> psum rows 0-63 / 64-127
    sel4 = np.zeros((4, 128, 128), np.float32)
    for jj in range(4):
        sel4[jj, 2 * jj, :64] = 1.0
        sel4[jj, 2 * jj + 1, 64:] = 1.0
    bf = ml_dtypes.bfloat16
    return (cos128.astype(bf), sin128.astype(bf), swp.astype(bf), umask2.astype(bf),
            selb.astype(bf), sel4.astype(bf))


def _shard_inputs(x, w_qkv, w_proj):
    """Per-core input dicts. Core i: batch i//4, group pair gp = i%4."""
    cos128, sin128, swp, umask2, selb, sel4 = _host_tables()
    bf = ml_dtypes.bfloat16
    # xt packed [128, 16, T]: xtp[p, ci, t] = x[b].T[128*ci+p, t]
    xtp = []
    for b in range(B):
        xt = x[b].T.astype(bf)  # [D, T]
        xtp.append(np.ascontiguousarray(xt.reshape(NCT, 128, T).transpose(1, 0, 2)))
    maps = []
    for i in range(NCORES):
        b, gp = i // 4, i % 4
        heads = [8 * gp + j for j in range(8)]  # global heads of this core
        # q blocks pair local heads (j, j+4) = (group 2gp head j, group 2gp+1 head j)
        qcols = []
        for j in range(4):
            qcols.append(w_qkv[:, 64 * heads[j] : 64 * heads[j] + 64])
            qcols.append(w_qkv[:, 64 * heads[j + 4] : 64 * heads[j + 4] + 64])
        kcol = w_qkv[:, D + 128 * gp : D + 128 * gp + 128]
        vcol = w_qkv[:, D + KV_DIM + 128 * gp : D + KV_DIM + 128 * gp + 128]
        wq = np.concatenate(qcols + [kcol, vcol], axis=1).astype(bf)  # [D, 768]
        wqp = np.ascontiguousarray(wq.reshape(NCT, 128, WCOLS).transpose(1, 0, 2))
        # w_proj rows in ypair order: pair j = [head j ; head j+4]
        wrows = []
        for j in range(4):
            wrows.append(w_proj[64 * heads[j] : 64 * heads[j] + 64, :])
            wrows.append(w_proj[64 * heads[j + 4] : 64 * heads[j + 4] + 64, :])
        wp = np.concatenate(wrows, axis=0).astype(bf)  # [512, D]
        wpp = np.ascontiguousarray(wp.reshape(4, 128, D).transpose(1, 0, 2))
        maps.append(
            {
                "xt": xtp[b],
                "wqkv": wqp,
                "wproj": wpp,
                "costab": cos128,
                "sintab": sin128,
                "swp": swp,
                "umask2": umask2,
                "selb": selb,
                "sel4": sel4,
            }
        )
    return maps


# ------------------------------------------------------------------- builder


def build_nc():
    nc = bacc.Bacc("TRN2", target_bir_lowering=False, debug=False, num_devices=NCORES)
    xt_d = nc.dram_tensor("xt", [128, NCT, T], BF16, kind="ExternalInput").ap()
    wq_d = nc.dram_tensor("wqkv", [128, NCT, WCOLS], BF16, kind="ExternalInput").ap()
    wp_d = nc.dram_tensor("wproj", [128, 4, D], BF16, kind="ExternalInput").ap()
    cos_d = nc.dram_tensor("costab", [128, T], BF16, kind="ExternalInput").ap()
    sin_d = nc.dram_tensor("sintab", [128, T], BF16, kind="ExternalInput").ap()
    swp_d = nc.dram_tensor("swp", [128, 128], BF16, kind="ExternalInput").ap()
    um2_d = nc.dram_tensor("umask2", [128, 2, 128], BF16, kind="ExternalInput").ap()
    slb_d = nc.dram_tensor("selb", [128, 128], BF16, kind="ExternalInput").ap()
    s4_d = nc.dram_tensor("sel4", [4, 128, 128], BF16, kind="ExternalInput").ap()
    out_d = nc.dram_tensor("out", [T, D], BF16, kind="ExternalOutput").ap()

    with (
        nc.allow_low_precision(reason="bf16 matmul operands; fp32 psum accumulation"),
        tile.TileContext(nc) as tc,
        ExitStack() as ctx,
    ):
        const = ctx.enter_context(tc.tile_pool(name="const", bufs=1))
        keep = ctx.enter_context(tc.tile_pool(name="keep", bufs=1))
        p_x = ctx.enter_context(tc.tile_pool(name="p_x", bufs=2))
        p_w = ctx.enter_context(tc.tile_pool(name="p_w", bufs=3))
        p_yh = ctx.enter_context(tc.tile_pool(name="p_yh", bufs=3))
        p_pt = ctx.enter_context(tc.tile_pool(name="p_pt", bufs=5))
        ps_a = ctx.enter_context(tc.tile_pool(name="ps_a", bufs=2, space="PSUM"))
        ps_sc = ctx.enter_context(tc.tile_pool(name="ps_sc", bufs=2, space="PSUM"))
        ps_pv = ctx.enter_context(tc.tile_pool(name="ps_pv", bufs=1, space="PSUM"))

        warm_t = const.tile([128, TQ], BF16)
        cos_t = const.tile([128, T], BF16)
        sin_t = const.tile([128, T], BF16)
        swp_t = const.tile([128, 128], BF16)
        um2_t = const.tile([128, 2, 128], BF16)
        slb_t = const.tile([128, 128], BF16)
        s4_t = const.tile([128, 4, 128], BF16)
        wq_sb = keep.tile([128, NCT, WCOLS], BF16, tag="wq", name="wq_sb")
        wp_sb = keep.tile([128, 4, D], BF16, tag="wp", name="wp_sb")

        # persistent per-chunk activations; q pairs head-stacked [h0; h1]
        qp_c = [
            [keep.tile([128, TQ], BF16, tag=f"qp{c}_{jp}", name=f"qp{c}_{jp}") for jp in range(4)]
            for c in range(NT)
        ]
        kp_c = [keep.tile([128, TQ], BF16, tag=f"kp{c}", name=f"kp{c}") for c in range(NT)]
        v_c = [keep.tile([128, TQ], BF16, tag=f"v{c}", name=f"v{c}") for c in range(NT)]
        va_c = [
            [keep.tile([128, 4, 65], BF16, tag=f"va{c}_{g}", name=f"va{c}_{g}") for g in range(2)]
            for c in range(NT)
        ]
        yp_c = [
            [keep.tile([128, TQ], BF16, tag=f"yp{c}_{jj}", name=f"yp{c}_{jj}") for jj in range(4)]
            for c in range(NT)
        ]
        sump_c = [keep.tile([128, TQ], BF16, tag=f"sump{c}", name=f"sump{c}") for c in range(NT)]

        # sump rows 8-127 are never written but are read by the bcast matmul
        # (times zero stationary cols); they must be finite. va cols 64 are the
        # static ones-columns (denominator trick); warm_t feeds warmup matmuls.
        nc.gpsimd.memset(warm_t[:], 0.0)
        for c in range(NT):
            nc.gpsimd.memset(sump_c[c][:], 1.0)
            for g in range(2):
                nc.gpsimd.memset(va_c[c][g][:, :, 64:65], 1.0)

        xts = [None] * NT  # xt sbuf tiles [128, NCT, TQ] per chunk

        # ---------------- work queue: qkv (hi) / proj (lo) matmul quanta
        q_hi = deque()
        q_lo = deque()

        def pull(n, lo_ok=True):
            for _ in range(n):
                if q_hi:
                    q_hi.popleft()()
                elif lo_ok and q_lo:
                    q_lo.popleft()()
                else:
                    break

        def drain_hi():
            while q_hi:
                q_hi.popleft()()

        # ---------------- DMA helpers (host-packed layouts, few big DMAs)
        def dma_xt(c, eng=None):
            xt_t = p_x.tile([128, NCT, TQ], BF16, tag="xt", name=f"xt{c}")
            (eng or nc.sync).dma_start(xt_t[:], xt_d[:, :, TQ * c : TQ * c + TQ])
            xts[c] = xt_t

        # ---------------- qkv + rope
        def rope_rest(c, raw, dest):
            ts = slice(TQ * c, TQ * c + TQ)
            sw = ps_a.tile([128, TQ], F32, tag="acc", name="sw")
            nc.tensor.matmul(sw[:], swp_t[:], raw[:], start=True, stop=True)
            t1 = p_w.tile([128, TQ], BF16, tag="t1", name="t1")
            t2 = p_w.tile([128, TQ], BF16, tag="t2", name="t2")
            nc.vector.tensor_mul(t1[:], raw[:], cos_t[:, ts])
            nc.vector.tensor_mul(t2[:], sw[:], sin_t[:, ts])
            nc.vector.tensor_add(dest[:], t1[:], t2[:])

        def qkv_mms(c, oc, lo, hi, ps):
            for ci in range(lo, hi):
                nc.tensor.matmul(
                    ps[:],
                    wq_sb[:, ci, 128 * oc : 128 * oc + 128],
                    xts[c][:, ci, :],
                    start=(ci == 0),
                    stop=(ci == NCT - 1),
                )

        def qkv_post(c, oc, hold):
            if oc < 5:
                rope_rest(c, hold["raw"], qp_c[c][oc] if oc < 4 else kp_c[c])
            else:
                for kloc in range(4):
                    vp = ps_a.tile([128, 128], F32, tag="acc", name="vp")
                    nc.tensor.matmul(
                        vp[:],
                        v_c[c][:, 128 * kloc : 128 * kloc + 128],
                        slb_t[:],
                        start=True,
                        stop=True,
                    )
                    nc.vector.tensor_copy(va_c[c][0][:, kloc, 0:64], vp[:, 0:64])
                    nc.vector.tensor_copy(va_c[c][1][:, kloc, 0:64], vp[:, 64:128])

        def enqueue_qkv(c):
            # per oc: 4 quanta of 4 accumulation mms (the last one also starts
            # the PSUM evacuation), then a separate quantum for the rope / v
            # post-processing so its PE ops (swap matmul, vp) land a quantum
            # later than the DVE copy they wait on. The psum tile is created
            # inside the FIRST quantum so ps_a slot-rotation order matches
            # emission order (no WAR deadlocks).
            for oc in [4, 5, 0, 1, 2, 3]:
                hold = {}
                for lo in range(0, NCT, 4):
                    def quant(c=c, oc=oc, lo=lo, hold=hold):
                        if lo == 0:
                            hold["ps"] = ps_a.tile(
                                [128, TQ], F32, tag="acc", name=f"qkv{c}_{oc}"
                            )
                        qkv_mms(c, oc, lo, lo + 4, hold["ps"])
                        if lo + 4 == NCT:
                            if oc == 5:
                                nc.vector.tensor_copy(v_c[c][:], hold["ps"][:])
                            else:
                                raw = p_w.tile([128, TQ], BF16, tag="raw", name="raw")
                                nc.vector.tensor_copy(raw[:], hold["ps"][:])
                                hold["raw"] = raw
                    q_hi.append(quant)
                q_hi.append(lambda c=c, oc=oc, hold=hold: qkv_post(c, oc, hold))

        # ---------------- attention
        def emit_sc(s, jp, kj):
            qcs, kloc = kj // 4, kj % 4
            col0 = max(kj * 128 - s * TQ, 0)
            sc = ps_sc.tile([128, 2, TQ], F32, tag="sc", name="sc")
            for h in range(2):
                nc.tensor.matmul(
                    sc[:, h, col0:TQ],
                    kp_c[qcs][64 * h : 64 * h + 64, 128 * kloc : 128 * kloc + 128],
                    qp_c[s][jp][64 * h : 64 * h + 64, col0:TQ],
                    start=True,
                    stop=True,
                )
            pt = p_pt.tile([128, 2, TQ], BF16, tag="pt", name="pt")
            nc.scalar.activation(
                pt[:, :, col0:TQ],
                sc[:, :, col0:TQ],
                mybir.ActivationFunctionType.Exp,
                scale=SCALE,
            )
            if kj >= 4 * s:  # diagonal tile: triangular keep-mask, both heads
                nc.gpsimd.tensor_mul(
                    pt[:, :, col0 : col0 + 128],
                    pt[:, :, col0 : col0 + 128],
                    um2_t[:],
                )
            return pt, col0

        def emit_pv(s, jp, kj, pv, pt, col0):
            qcs, kloc = kj // 4, kj % 4
            nkj = 4 * s + 4
            for h in range(2):
                nc.tensor.matmul(
                    pv[0:65, h, col0:TQ],
                    va_c[qcs][h][:, kloc, :],
                    pt[:, h, col0:TQ],
                    start=(kj == 0),
                    stop=(kj == nkj - 1),
                )

        def attn_evac(s, jp, pv):
            # y of h0 -> yp rows 0-63 directly; h1 via staging + partition-move
            # DMA; both denominators in one copy + one DMA into sump rows.
            nc.vector.tensor_copy(yp_c[s][jp][0:64, :], pv[0:64, 0, :])
            yh = p_yh.tile([64, TQ], BF16, tag="yh", name="yh")
            nc.vector.tensor_copy(yh[0:64, :], pv[0:64, 1, :])
            nc.sync.dma_start(yp_c[s][jp][64:128, :], yh[0:64, :])
            dn = p_yh.tile([65, 2, TQ], BF16, tag="dn", name="dn")
            nc.vector.tensor_copy(dn[64:65, :, :], pv[64:65, :, :])
            nc.sync.dma_start(sump_c[s][2 * jp : 2 * jp + 2, :], dn[64:65, :, :])

        cr = [0.0]  # fractional filler-pull credit, persists across jp

        def attn(s, jp, pace, lo_ok):
            nkj = 4 * s + 4
            pv = ps_pv.tile([128, 2, TQ], F32, tag="pv", name="pv")

            def paced_pull():
                cr[0] += pace
                n = int(cr[0])
                cr[0] -= n
                pull(n, lo_ok)

            prev = emit_sc(s, jp, 0)
            for kj in range(1, nkj):
                cur = emit_sc(s, jp, kj)
                paced_pull()
                emit_pv(s, jp, kj - 1, pv, *prev)
                prev = cur
            paced_pull()
            emit_pv(s, jp, nkj - 1, pv, *prev)
            attn_evac(s, jp, pv)

        # ---------------- normalization + projection
        def norm_recip(c):
            nc.vector.reciprocal(sump_c[c][0:8, :], sump_c[c][0:8, :])

        def norm_bcast(c):
            for jj in range(4):
                bc = ps_a.tile([128, TQ], F32, tag="acc", name="bc")
                nc.tensor.matmul(
                    bc[:], s4_t[:, jj, :], sump_c[c][:], start=True, stop=True
                )
                nc.vector.tensor_mul(yp_c[c][jj][:], yp_c[c][jj][:], bc[:])

        def proj_unit(c, tb, oc, hold):
            if oc == 0:
                hold["ot"] = p_w.tile(
                    [128, D], BF16, tag="ot", name=f"ot{c}_{tb}", bufs=3
                )
            ot = hold["ot"]
            pj = ps_a.tile([128, TQ], F32, tag="acc", name="pj")
            for jj in range(4):
                nc.tensor.matmul(
                    pj[:],
                    yp_c[c][jj][:, 128 * tb : 128 * tb + 128],
                    wp_sb[:, jj, TQ * oc : TQ * oc + TQ],
                    start=(jj == 0),
                    stop=(jj == 3),
                )
            tok0 = c * TQ + tb * 128
            if c == NT - 1:
                # epilogue: ACT is idle -> alternate evac engines, store per oc
                # so the out DMAs drain while the remaining proj mms still run
                if oc % 2 == 1:
                    nc.scalar.copy(ot[:, TQ * oc : TQ * oc + TQ], pj[:])
                else:
                    nc.vector.tensor_copy(ot[:, TQ * oc : TQ * oc + TQ], pj[:])
                nc.sync.dma_start(
                    out_d[tok0 : tok0 + 128, TQ * oc : TQ * oc + TQ],
                    ot[:, TQ * oc : TQ * oc + TQ],
                )
            else:
                nc.vector.tensor_copy(ot[:, TQ * oc : TQ * oc + TQ], pj[:])
                if oc == 3:
                    nc.sync.dma_start(out_d[tok0 : tok0 + 128, :], ot[:])

        def enqueue_proj(c):
            for tb in range(4):
                hold = {}
                for oc in range(4):
                    q_lo.append(
                        lambda c=c, tb=tb, oc=oc, hold=hold: proj_unit(c, tb, oc, hold)
                    )

        # ---------------- emission
        # warmup matmuls: un-throttle the HAM clock during the initial DMA wait
        wps = ps_a.tile([128, TQ], F32, tag="acc", name="warm")
        for wi in range(8):
            nc.tensor.matmul(
                wps[:], warm_t[:, 0:128], warm_t[:], start=(wi == 0), stop=(wi == 7)
            )

        # critical DMAs first: wq + xt(0) interleaved in eighths, so each PE
        # wait during the DMA-bound ramp stays well under the ~3.4us HAM
        # re-throttle window
        for g in range(8):
            nc.sync.dma_start(wq_sb[:, 2 * g : 2 * g + 2, :], wq_d[:, 2 * g : 2 * g + 2, :])
            xt_t = (
                p_x.tile([128, NCT, TQ], BF16, tag="xt", name="xt0") if g == 0 else xts[0]
            )
            xts[0] = xt_t
            nc.sync.dma_start(
                xt_t[:, 2 * g : 2 * g + 2, :], xt_d[:, 2 * g : 2 * g + 2, 0:TQ]
            )
        dma_xt(1)
        nc.sync.dma_start(cos_t[:], cos_d)
        nc.sync.dma_start(sin_t[:], sin_d)
        nc.sync.dma_start(swp_t[:], swp_d)
        nc.sync.dma_start(um2_t[:], um2_d)
        nc.sync.dma_start(slb_t[:], slb_d)
        for jj in range(4):
            nc.sync.dma_start(s4_t[:, jj, :], s4_d[jj, :, :])
        nc.sync.dma_start(wp_sb[:], wp_d)

        # qkv(0) runs ci-major in oc-pairs: each arriving (wq, xt) eighth
        # unlocks 4 matmuls, so the DMA-paced ramp has no long PE gaps
        for ocA, ocB in [(4, 5), (0, 1), (2, 3)]:
            psA = ps_a.tile([128, TQ], F32, tag="acc", name=f"qkv0_{ocA}")
            psB = ps_a.tile([128, TQ], F32, tag="acc", name=f"qkv0_{ocB}")
            for ci in range(NCT):
                for oc, ps in ((ocA, psA), (ocB, psB)):
                    nc.tensor.matmul(
                        ps[:],
                        wq_sb[:, ci, 128 * oc : 128 * oc + 128],
                        xts[0][:, ci, :],
                        start=(ci == 0),
                        stop=(ci == NCT - 1),
                    )
            for oc, ps in ((ocA, psA), (ocB, psB)):
                hold = {"ps": ps}
                if oc == 5:
                    nc.vector.tensor_copy(v_c[0][:], ps[:])
                else:
                    raw = p_w.tile([128, TQ], BF16, tag="raw", name="raw")
                    nc.vector.tensor_copy(raw[:], ps[:])
                    hold["raw"] = raw
                qkv_post(0, oc, hold)

        for s in range(NT):
            drain_hi()  # qkv(s) leftovers must finish before attn(s)
            if s + 2 < NT:
                dma_xt(s + 2)  # after drain: its p_x slot's old readers are emitted
            if s + 1 < NT:
                enqueue_qkv(s + 1)
            if s >= 1:
                norm_recip(s - 1)
            # filler supply this window: qkv(s+1) quanta (+ proj(s-1) units,
            # except in window 2, where proj is deferred to window 3 to match
            # window 3's large exp batch); spread evenly over attention tiles
            lo_ok = s != 2
            navail = len(q_hi) + len(q_lo)
            if s >= 1 and lo_ok:
                navail += 16  # proj(s-1), enqueued after attn(s, 0)
            if not lo_ok:
                navail = len(q_hi)
            if s == NT - 1:
                navail = max(0, navail - 6)  # reserve PE work for the recip gap
            pace = navail / (4 * (4 * s + 4))
            for jp in range(4):
                attn(s, jp, pace, lo_ok)
                if s >= 1 and jp == 0:
                    norm_bcast(s - 1)
                    enqueue_proj(s - 1)
        norm_recip(NT - 1)
        while q_lo:
            q_lo.popleft()()
        norm_bcast(NT - 1)
        enqueue_proj(NT - 1)
        while q_lo:
            q_lo.popleft()()

    nc.compile()
    return nc


_NC_CACHE = None


def _get_nc():
    global _NC_CACHE
    if _NC_CACHE is None:
        _NC_CACHE = build_nc()
    return _NC_CACHE


def kernel(x, w_qkv, w_proj, _trace=False, _nc=None):
    x = np.asarray(x, np.float32)
    w_qkv = np.asarray(w_qkv, np.float32)
    w_proj = np.asarray(w_proj, np.float32)
    nc = _nc if _nc is not None else _get_nc()
    in_maps = _shard_inputs(x, w_qkv, w_proj)
    res = run_bass_kernel_spmd(nc, in_maps, core_ids=list(range(NCORES)), trace=_trace)
    out = np.zeros((B, T, D), np.float32)
    for i in range(NCORES):
        out[i // 4] += res.results[i]["out"].astype(np.float32)
    if _trace:
        return out, res
    return out


if __name__ == "__main__":
    rng = np.random.default_rng(0)
    x = rng.standard_normal((B, T, D), dtype=np.float32)
    wq = rng.standard_normal((D, D + 2 * KV_DIM), dtype=np.float32) * D**-0.5
    wp = rng.standard_normal((D, D), dtype=np.float32) * D**-0.5
    y = kernel(x, wq, wp)
    print(y.shape, y.dtype)
